# revision 72
# baseline (speedup 1.0000x reference)
"""Trainium2 Bass kernel for nn_AttentionAggregate_Weight (gnn_message_passing).

Computes, per node n with K=32 neighbors and D=128 features:
    score[n,k] = tanh(nodes_key[n].v1 + middle_key[n,k].v2 + a_b)
    out[n,:]   = sum_k softmax_k(score)[n,k] * middle_value[n,k,:]
where v1 = W1.T @ a_w and v2 = W2.T @ a_w are folded on the host (the
reference's p1/p2 projections only ever appear dotted with a_w; tanh
outputs lie in [-1,1] so the softmax needs no max subtraction and the
denominator folds into one final per-node scale).

Distribution: pure data parallel over the node axis across 8 NeuronCores.
Nodes are host-padded 20000 -> 20480 so each core gets 2560 = 20 tiles of
exactly 128 nodes. middle_key streams as host-pretransposed fp8_e3m4
tile-major [d, (k, n)] slabs; middle_value is bf16 on the two HWDGE
rings (value precision directly bounds output error) except the SWDGE
share, which ships fp8_e3m4 and is cast to bf16 during the DMA (only
SWDGE can cast); nodes_key is bf16, host-transposed [d, n], loaded in
one up-front 640 KB contiguous DMA. Total HBM traffic 29.6 MB/core vs
the 43.25 MB bf16 baseline; final rel-err ~1.1e-2 vs the 2e-2 gate.

Engine split per 128-node tile (the bf16 baseline was VectorE-bound at
~4.5 us/tile of MACs, so every non-MAC op is pushed elsewhere):

- TensorE builds the pre-tanh score tile TRANSPOSED [k, n] in one PSUM
  accumulation group: one matmul (stationary = v1 replicated K columns,
  moving = the host-transposed nk slab) broadcasts the center-node term,
  then K matmuls (stationary = tiny constant one-hot*v2 [d, K] slices,
  moving = the fp8 mkT stream). With the 10.5 MB mk stream on the MOVING
  side, PE pays no per-tile LDWEIGHTS on it (the node-major orientation
  would cost ~3.5 us/tile of 128-column stationary loads). A PE
  transpose against a 32x32 identity returns exp's output to node-major.
- ScalarE: tanh (a_b via the bias port), exp, softmax normalization
  (accum_out row-sum + per-partition 1/Z scale port), plus the first
  N_ACT_HEAD value-chain head products (scale port = coef_k).
- VectorE: only the reciprocal and the fused bf16 multiply-accumulate
  chains (2x-packed scalar_tensor_tensor), chains interleaved in
  emission so same-accumulator ops never run back-to-back, plus the
  final merge which writes the output tile directly.
- GpSimd: first-level chain merges (tensor_add), the SWDGE share of the
  middle_value load, and the output store — it has no fused MAC in this
  backend and its Q7 software ops run ~2.6 cyc/elem, so it gets adds,
  not MACs.
- DMA: middle_key + nodes_key + 14 middle_value k's on the SP HWDGE
  ring, 10 middle_value k's on the ACT HWDGE ring, 8 (as fp8, cast
  in-flight) on GpSimd SWDGE — keeping the ACT ring light so ScalarE's
  compute share can grow.

The tile loop is software-pipelined (SKEW=1 with multi-buffered pools);
outputs return as bf16 and are upcast on the host.

Self-contained: hardcodes shapes/sharding; no file I/O.
"""

from contextlib import ExitStack

import numpy as np

N, K, D = 20000, 32, 128
N_CORES = 8
NPC = 2560  # padded nodes per core (20 tiles of 128)
NPAD = NPC * N_CORES
P = 128
SKEW = 1
BUFS = 3
MK_FP8 = True  # stage middle_key as fp8_e3m4 (scores only; ~7.5e-3 rel err)
KGP = 0  # value-chain k's on GpSimd (Q7 SW ops ~4x slower than DVE: keep 0)
NSUB_D = 8  # value sub-chains (heads run on ScalarE, MACs on VectorE)
NSUB_G = 2  # GpSimd sub-chains (unused when KGP=0)
SCORES_T = True  # flipped score matmuls: one-hot*v2 stationaries, mk moving
MV_SYNC = 14  # trailing middle_value k's loaded on the SP HWDGE ring
MV_GP = 8  # middle_value k's loaded via GpSimd SWDGE
OUT_Q = "gpsimd"  # output store queue
N_POOL_MERGE = 6  # first-level chain merges on GpSimd (relieves VectorE)
N_ACT_HEAD = 8  # chain heads on ScalarE (rest on VectorE's 4x ts_mul)


# ---------------------------------------------------------------------------
# Wait legalization: this walrus build accepts at most ONE semaphore wait per
# instruction; split extras onto same-engine Drain carriers at the BIR level.
# ---------------------------------------------------------------------------
def _legalize_bir_waits(bir_bytes: bytes) -> bytes:
    import orjson

    m = orjson.loads(bir_bytes)
    n = 0
    for f in m.get("functions", []):
        for b in f.get("blocks", []):
            insts = b.get("instructions") or []
            out = []
            changed = False
            for ins in insts:
                si = ins.get("sync_info")
                waits = (si or {}).get("on_wait") or []
                if len(waits) > 1:
                    changed = True
                    for w in waits[:-1]:
                        n += 1
                        out.append(
                            {
                                "debug": ins.get("debug", 0),
                                "engine": ins.get("engine"),
                                "ins": [],
                                "outs": [],
                                "name": f"I-wfix-{n}",
                                "opcode": "Drain",
                                "sync_info": {"on_update": [], "on_wait": [w]},
                            }
                        )
                    si["on_wait"] = [waits[-1]]
                out.append(ins)
            if changed:
                b["instructions"] = out
    return orjson.dumps(m)


_waitfix_installed = False


def _install_waitfix():
    global _waitfix_installed
    if _waitfix_installed:
        return
    import concourse.bass as bass

    orig = bass.Bass.to_json_bytes

    def patched(self):
        return _legalize_bir_waits(orig(self))

    bass.Bass.to_json_bytes = patched
    _waitfix_installed = True


def _chains(ks, nsub):
    """Split k-indices into nsub round-robin-balanced contiguous chains."""
    if not ks:
        return []
    base, extra = divmod(len(ks), nsub)
    out = []
    pos = 0
    for c in range(nsub):
        ln = base + (1 if c < extra else 0)
        out.append(ks[pos : pos + ln])
        pos += ln
    return [c for c in out if c]


# ---------------------------------------------------------------------------
# Kernel builder (per-core: NPC nodes, P=128 per tile)
# ---------------------------------------------------------------------------
def _build_kernel(repeat=1, load_frac=1.0, dve_fd=None):
    import concourse.bass as bass
    import concourse.tile as tile
    from concourse import mybir

    f32 = mybir.dt.float32
    bf16 = mybir.dt.bfloat16
    mk_dt = mybir.dt.float8e3 if MK_FP8 else bf16
    n_tiles = NPC // P

    nc = bass.Bass()
    # host-pretransposed, tile-major: row block t*D..(t+1)*D is tile t's
    # [d, (k, p)] slab (k-major free so per-k stationary slices are contiguous)
    mk = nc.dram_tensor("mk", (n_tiles * D, P * K), mk_dt, kind="ExternalInput")
    # nodes_key, host-transposed to [d, n] (contiguous per-partition rows)
    nk = nc.dram_tensor("nk", (D, NPC), bf16, kind="ExternalInput")
    mv = nc.dram_tensor("mv", (NPC, K, D), bf16, kind="ExternalInput")
    # the SWDGE share of middle_value ships as fp8_e3m4 and is cast to
    # bf16 during the DMA (HWDGE can't cast; SWDGE can) — 8% less traffic
    mv8 = nc.dram_tensor(
        "mv8", (NPC, MV_GP or 1, D), mybir.dt.float8e3, kind="ExternalInput"
    )
    # cb: col 0 = v2; cols 1..1+K = v1 replicated; then K one-hot*v2
    # stationaries zk[d, (k, j)] = v2[d]*(j==k); then a 32x32 identity
    cb_d = nc.dram_tensor(
        "cb", (128, 1 + K + K * K + 32), bf16, kind="ExternalInput"
    )
    cf_d = nc.dram_tensor("cf", (128, 1), f32, kind="ExternalInput")
    out = nc.dram_tensor("out", (NPC, D), bf16, kind="ExternalOutput")

    with tile.TileContext(nc) as tc, ExitStack() as ctx:
        singles = ctx.enter_context(tc.tile_pool(name="singles", bufs=1))
        keys = ctx.enter_context(tc.tile_pool(name="keys", bufs=BUFS + SKEW))
        vals = ctx.enter_context(tc.tile_pool(name="vals", bufs=BUFS + SKEW))
        nks = ctx.enter_context(tc.tile_pool(name="nks", bufs=BUFS))
        outs = ctx.enter_context(tc.tile_pool(name="outs", bufs=BUFS))
        smalls = ctx.enter_context(tc.tile_pool(name="smalls", bufs=BUFS + SKEW))
        accs = ctx.enter_context(tc.tile_pool(name="accs", bufs=3))
        psums = ctx.enter_context(tc.tile_pool(name="psums", bufs=3, space="PSUM"))

        cb = singles.tile([128, 1 + K + K * K + 32], bf16)
        nc.gpsimd.dma_start(out=cb, in_=cb_d[:])
        cf = singles.tile([128, 1], f32)
        nc.gpsimd.dma_start(out=cf, in_=cf_d[:])
        v2col = cb[:, 0:1]
        v1rep = cb[:, 1 : 1 + K]  # v1 replicated K columns
        zk3 = cb[:, 1 + K : 1 + K + K * K].rearrange("d (k j) -> d k j", j=K)
        id32 = cb[0:32, 1 + K + K * K :]
        ab_sb = cf[0:P, 0:1]
        # nodes_key is tiny (640 KB): load every tile's [d, n] slab in ONE
        # DMA up front (per-tile 256 B/partition lines are descriptor-bound)
        nk_all = singles.tile([D, NPC], bf16)
        nc.sync.dma_start(out=nk_all, in_=nk[:])
        # dummy touches: engines observe the const-DMA semaphores up front
        dum = singles.tile([1, 2], f32)
        nc.vector.tensor_copy(out=dum[:, 0:1], in_=cf[0:1, 0:1])
        nc.scalar.activation(
            out=dum[:, 1:2], in_=cb[0:1, 0:1],
            func=mybir.ActivationFunctionType.Copy,
        )

        mk_cols = int(P * K * load_frac)  # <1.0: timing-probe variants only
        k_sc = K - MV_SYNC - MV_GP  # mv k's on the scalar (ACT) ring

        def emit_loads(i, t):
            rows = slice(t * P, (t + 1) * P)
            mkT = keys.tile([D, P * K], mk_dt, tag="mkT", name=f"mkT_{i}")
            nc.sync.dma_start(
                out=mkT[:, :mk_cols], in_=mk[t * D : (t + 1) * D, :mk_cols]
            )
            val3 = vals.tile([P, K, D], bf16, tag="val3", name=f"val3_{i}")
            kv = max(1, int(k_sc * load_frac))
            nc.scalar.dma_start(out=val3[:, :kv, :], in_=mv[rows, :kv, :])
            if MV_SYNC:
                nc.sync.dma_start(
                    out=val3[:, k_sc : k_sc + MV_SYNC, :],
                    in_=mv[rows, k_sc : k_sc + MV_SYNC, :],
                )
            if MV_GP:
                nc.gpsimd.dma_start(
                    out=val3[:, k_sc + MV_SYNC :, :], in_=mv8[rows]
                )
            return {"mkT": mkT, "val3": val3, "nkT": nk_all[:, t * P : (t + 1) * P]}

        def emit_scores_T(i, h):
            # flipped orientation: transposed score tile [k, n] built in one
            # PSUM accumulation group; stationaries are tiny constants (v1rep
            # and per-k one-hot*v2), the big mk data streams as the MOVING
            # operand, so PE pays no per-tile LDWEIGHTS on the 10.5 MB stream
            ps = psums.tile([K, P], f32, tag="ps", name=f"ps_{i}")
            nc.tensor.matmul(ps, v1rep, h["nkT"], start=True, stop=False)
            mkT3 = h["mkT"].rearrange("d (k p) -> d k p", p=P)
            for k in range(K):
                nc.tensor.matmul(
                    ps, zk3[:, k, :], mkT3[:, k, :],
                    start=False, stop=(k == K - 1),
                )
            thT = smalls.tile([K, P], bf16, tag="thT", name=f"thT_{i}")
            nc.scalar.activation(
                out=thT, in_=ps, func=mybir.ActivationFunctionType.Tanh,
                bias=cf[0:K, 0:1], scale=1.0,
            )
            eT = smalls.tile([K, P], bf16, tag="eT", name=f"eT_{i}")
            nc.scalar.activation(
                out=eT, in_=thT, func=mybir.ActivationFunctionType.Exp
            )
            h["eT"] = eT

        def emit_scores_post(i, h):
            # emitted one iteration AFTER emit_scores_T: PE/ACT are strict
            # FIFO, so placing the transpose (waits on exp) and the
            # normalization chain behind the NEXT tile's score matmuls keeps
            # those engines streaming instead of head-of-line blocking
            e4 = psums.tile([P, K], bf16, tag="e4", name=f"e4_{i}")
            nc.tensor.matmul(e4, h["eT"], id32, is_transpose=True)
            ecp = smalls.tile([P, K], bf16, tag="ecp", name=f"ecp_{i}")
            sums = smalls.tile([P, 1], f32, tag="sums", name=f"sums_{i}")
            nc.scalar.activation(
                out=ecp, in_=e4, func=mybir.ActivationFunctionType.Copy,
                accum_out=sums,
            )
            recip = smalls.tile([P, 1], f32, tag="recip", name=f"recip_{i}")
            nc.vector.reciprocal(out=recip, in_=sums)
            coef = smalls.tile([P, K], f32, tag="coef", name=f"coef_{i}")
            nc.scalar.activation(
                out=coef, in_=ecp, func=mybir.ActivationFunctionType.Copy,
                scale=recip,
            )
            h["coef"] = coef

        def emit_scores_N(i, h):
            # node-major orientation: s1 broadcast + K neighbor dots, one
            # accumulation group; stationary = the mk stream (LDWEIGHTS-heavy)
            ps = psums.tile([P, K], f32, tag="ps", name=f"ps_{i}")
            nc.tensor.matmul(ps[:, 0:K], h["nkT"], v1rep, start=True, stop=False)
            mkT3 = h["mkT"].rearrange("d (k p) -> d k p", p=P)
            for k in range(K):
                nc.tensor.matmul(
                    ps[:, k : k + 1], mkT3[:, k, :], v2col,
                    start=False, stop=(k == K - 1), skip_group_check=True,
                )
            th = smalls.tile([P, K], bf16, tag="th", name=f"th_{i}")
            nc.scalar.activation(
                out=th, in_=ps, func=mybir.ActivationFunctionType.Tanh,
                bias=ab_sb, scale=1.0,
            )
            e_t = smalls.tile([P, K], f32, tag="e_t", name=f"e_{i}")
            sums = smalls.tile([P, 1], f32, tag="sums", name=f"sums_{i}")
            nc.scalar.activation(
                out=e_t, in_=th, func=mybir.ActivationFunctionType.Exp,
                accum_out=sums,
            )
            recip = smalls.tile([P, 1], f32, tag="recip", name=f"recip_{i}")
            nc.vector.reciprocal(out=recip, in_=sums)
            coef = smalls.tile([P, K], f32, tag="coef", name=f"coef_{i}")
            nc.vector.tensor_scalar_mul(out=coef, in0=e_t, scalar1=recip)
            h["coef"] = coef

        emit_scores = emit_scores_T if SCORES_T else emit_scores_N

        vfd = dve_fd or D  # <D: timing-probe variants only
        dve_ks = _chains(list(range(K - KGP)), NSUB_D)
        gp_ks = _chains(list(range(K - KGP, K)), NSUB_G)
        out_eng = {"sync": nc.sync, "scalar": nc.scalar, "gpsimd": nc.gpsimd}[
            OUT_Q
        ]

        def emit_values(i, t, h):
            val3, coef = h["val3"], h["coef"]
            rows = slice(t * P, (t + 1) * P)
            # out_t = sum_k val_k * coef_k (coef pre-normalized): bf16 MAC
            # sub-chains. Chain heads are plain products and run on ScalarE's
            # per-partition scale port; VectorE does only the fused MACs;
            # GpSimd takes the first merge level. Chains interleave in
            # emission so same-accumulator ops never run back-to-back.
            chains = [(nc.vector, f"d{c}", ks, vfd) for c, ks in enumerate(dve_ks)]
            chains += [(nc.gpsimd, f"g{c}", ks, D) for c, ks in enumerate(gp_ks)]
            acc_of = {}
            for eng, cn, ks, fd in chains:
                acc_of[cn] = accs.tile(
                    [P, D], bf16, tag=f"acc_{cn}", name=f"acc{cn}_{i}"
                )
            depth = max(len(ks) for _, _, ks, _ in chains)
            for step in range(depth):
                for eng, cn, ks, fd in chains:
                    if step >= len(ks):
                        continue
                    k = ks[step]
                    acc = acc_of[cn]
                    if step == 0:
                        # head products: first N_ACT_HEAD chains start on
                        # ScalarE (scale port = coef_k), the rest on VectorE
                        if int(cn[1:]) < N_ACT_HEAD and eng is nc.vector:
                            nc.scalar.activation(
                                out=acc[:, :fd], in_=val3[:, k, :fd],
                                func=mybir.ActivationFunctionType.Copy,
                                scale=coef[:, k : k + 1],
                            )
                        else:
                            eng.tensor_scalar_mul(
                                out=acc[:, :fd], in0=val3[:, k, :fd],
                                scalar1=coef[:, k : k + 1],
                            )
                    elif eng is nc.vector:
                        eng.scalar_tensor_tensor(
                            out=acc[:, :fd], in0=val3[:, k, :fd],
                            scalar=coef[:, k : k + 1], in1=acc[:, :fd],
                            op0=mybir.AluOpType.mult, op1=mybir.AluOpType.add,
                        )
                    else:
                        # Pool has no fused MAC in this backend: mul + add
                        prod = accs.tile(
                            [P, D], bf16, tag=f"gp_{cn}", name=f"gp{cn}_{i}_{k}"
                        )
                        eng.tensor_scalar_mul(
                            out=prod, in0=val3[:, k, :],
                            scalar1=coef[:, k : k + 1],
                        )
                        eng.tensor_add(out=acc, in0=acc, in1=prod)
            # merge tree: first level on GpSimd (up to N_POOL_MERGE adds),
            # the rest on VectorE; the last merge writes the output tile
            sub = [acc_of[cn] for _, cn, _, _ in chains]
            out_t = outs.tile([P, D], bf16, tag="out_t", name=f"out_{i}")
            pool_left = N_POOL_MERGE
            while len(sub) > 1:
                nxt = []
                for c2 in range(0, len(sub) - 1, 2):
                    if len(sub) == 2:
                        dst = out_t
                    else:
                        dst = accs.tile(
                            [P, D], bf16, tag=f"cmb{len(sub)}_{c2}",
                            name=f"cmb{len(sub)}_{c2}_{i}",
                        )
                    if pool_left > 0 and len(sub) > 2:
                        nc.gpsimd.tensor_add(out=dst, in0=sub[c2], in1=sub[c2 + 1])
                        pool_left -= 1
                    else:
                        nc.vector.tensor_add(out=dst, in0=sub[c2], in1=sub[c2 + 1])
                    nxt.append(dst)
                if len(sub) % 2:
                    nxt.append(sub[-1])
                sub = nxt
            if sub[0] is not out_t:
                nc.vector.tensor_copy(out=out_t, in_=sub[0])
            out_eng.dma_start(out=out[rows], in_=out_t)

        handles = {}
        n_iters = repeat * n_tiles
        for i in range(n_iters + SKEW):
            if i < n_iters:
                h = emit_loads(i, i % n_tiles)
                emit_scores(i, h)
                handles[i] = h
            if SCORES_T and i < n_iters:
                emit_scores_post(i, h)
            j = i - SKEW
            if j >= 0:
                emit_values(j, j % n_tiles, handles.pop(j))

    nc.finalize()
    return nc


_nc_cache = {}


def _get_nc():
    if "main" not in _nc_cache:
        _install_waitfix()
        _nc_cache["main"] = _build_kernel()
    return _nc_cache["main"]


def _host_prep(W1, W2, a_w, a_b):
    import ml_dtypes

    v1 = (W1.astype(np.float64).T @ a_w.astype(np.float64)).astype(np.float32)
    v2 = (W2.astype(np.float64).T @ a_w.astype(np.float64)).astype(np.float32)
    cb = np.zeros((128, 1 + K + K * K + 32), np.float32)
    cb[:, 0] = v2
    cb[:, 1 : 1 + K] = v1[:, None]
    zk = np.zeros((128, K, K), np.float32)
    zk[:, np.arange(K), np.arange(K)] = v2[:, None]
    cb[:, 1 + K : 1 + K + K * K] = zk.reshape(128, K * K)
    cb[:32, 1 + K + K * K :] = np.eye(32, dtype=np.float32)
    cf = np.full((128, 1), np.float32(a_b[0]), np.float32)
    return cb.astype(ml_dtypes.bfloat16), cf


def host_mkT(mk_pad, npc=NPC):
    """[NPAD,K,D] fp32 -> tile-major [(tile,d), (k,p)] slabs (fp8/bf16)."""
    import ml_dtypes

    dt = ml_dtypes.float8_e3m4 if MK_FP8 else ml_dtypes.bfloat16
    npad = mk_pad.shape[0]
    return np.ascontiguousarray(
        mk_pad.reshape(npad // P, P, K, D).transpose(0, 3, 2, 1).astype(dt)
    ).reshape(-1, P * K)


def host_nkT(nk_pad):
    """[NPAD,D] -> per-core [d, n] bf16 (stacked (core*D, NPC))."""
    import ml_dtypes

    return np.ascontiguousarray(
        nk_pad.reshape(N_CORES, NPC, D)
        .transpose(0, 2, 1)
        .astype(ml_dtypes.bfloat16)
    ).reshape(N_CORES * D, NPC)


def kernel(middle_key, nodes_key, middle_value, W1, W2, a_w, a_b):
    import ml_dtypes

    bf = ml_dtypes.bfloat16
    # middle_key: pad, then cast+transpose into tile-major [d, (k, p)] slabs
    mk_pad = np.zeros((NPAD, K, D), np.float32)
    mk_pad[:N] = np.ascontiguousarray(middle_key, np.float32)
    mkb = host_mkT(mk_pad)
    nk_pad = np.zeros((NPAD, D), np.float32)
    nk_pad[:N] = np.ascontiguousarray(nodes_key, np.float32)
    nkb = host_nkT(nk_pad)
    mvb = np.zeros((NPAD, K, D), bf)
    mvb[:N] = np.ascontiguousarray(middle_value, np.float32).astype(bf)
    k8 = K - MV_GP
    mv8b = np.zeros((NPAD, max(MV_GP, 1), D), ml_dtypes.float8_e3m4)
    if MV_GP:
        mv8b[:N] = np.ascontiguousarray(
            middle_value[:, k8:, :], np.float32
        ).astype(ml_dtypes.float8_e3m4)
    cb, cf = _host_prep(W1, W2, a_w, a_b)

    nc = _get_nc()

    rpc = mkb.shape[0] // N_CORES
    in_maps = []
    for c in range(N_CORES):
        s = slice(c * NPC, (c + 1) * NPC)
        sk = slice(c * rpc, (c + 1) * rpc)
        sn = slice(c * D, (c + 1) * D)
        in_maps.append(
            {
                "mk": mkb[sk],
                "nk": nkb[sn],
                "mv": mvb[s],
                "mv8": mv8b[s],
                "cb": cb,
                "cf": cf,
            }
        )

    from concourse import bass2jax

    results = bass2jax.run_bass_via_pjrt(nc, in_maps, n_cores=N_CORES)
    full = np.concatenate([r["out"] for r in results], axis=0)
    return full[:N].astype(np.float32)
